# revision 2
# baseline (speedup 1.0000x reference)
"""Single-head causal attention on 8 TRN2 NeuronCores — v2 (pipelined load).

Problem: x[B=8, T=2048, C=1024], Wq/Wk/Wv[C, H=64] (fp32)
  q = x@Wq; k = x@Wk; v = x@Wv
  wei = softmax(mask(q k^T * C^-0.5)); out = wei @ v       -> [B, T, H]

Sharding: data-parallel over batch, one batch element per core.

v2 changes vs baseline:
  - x loaded fp32 in 16 one-t-tile pieces on the scalar HWDGE queue,
    cast to bf16 on DVE (2x mode) as pieces land, xbar-transposed on the
    sync queue -> QKV chunk 0 starts at ~8us instead of ~35us.
  - S^T row-packed: even s-blocks at partitions 0:64 (kT copy), odd
    s-blocks at partitions 64:128 (kT in situ in qk_a + duplicated qT)
    -> 2 concurrent K=64 matmuls per pair.
  - exp batched per pair [128,1024] across 2 PSUM banks (off-diag).
  - causal mask applied as post-exp affine_select zeroing on bf16 pt
    (gpsimd) instead of pre-exp additive mask on PSUM (DVE).
  - out stores + x loads on the scalar queue (safe concurrent w/ xbar).
"""
import sys

sys.path.insert(0, "/opt/trn_rl_repo")

import numpy as np

import concourse.bass as bass
import concourse.mybir as mybir
import concourse.tile as tile
from concourse import bacc
from concourse.bass_utils import run_bass_kernel_spmd
from concourse.masks import make_identity

B, T, C, H = 8, 2048, 1024, 64
NTT = T // 128   # 16 t-tiles
NCT = C // 128   # 8  c-tiles
NCH = T // 512   # 4  t-chunks
SCALE = float(C) ** -0.5
VP = 80          # v_nat per-tile stride: 160B, 32B-aligned for xbar transpose

F32 = mybir.dt.float32
BF16 = mybir.dt.bfloat16


def build_nc(reps=1):
    nc = bacc.Bacc("TRN2", target_bir_lowering=False, debug=False)
    xD = nc.dram_tensor("x", [T, C], F32, kind="ExternalInput").ap()
    wqD = nc.dram_tensor("Wq", [C, H], F32, kind="ExternalInput").ap()
    wkD = nc.dram_tensor("Wk", [C, H], F32, kind="ExternalInput").ap()
    wvD = nc.dram_tensor("Wv", [C, H], F32, kind="ExternalInput").ap()
    outD = nc.dram_tensor("out", [T, H], F32, kind="ExternalOutput").ap()

    AF = mybir.ActivationFunctionType
    ALU = mybir.AluOpType

    with tile.TileContext(nc) as tc:
        with (
            tc.tile_pool(name="const", bufs=1) as cpool,
            tc.tile_pool(name="xnat", bufs=1) as xnpool,
            tc.tile_pool(name="xt", bufs=1) as xtpool,
            tc.tile_pool(name="qk", bufs=1) as qkpool,
            tc.tile_pool(name="pt", bufs=4) as ptpool,
            tc.tile_pool(name="osb", bufs=3) as opool,
            tc.tile_pool(name="fin", bufs=2) as fpool,
        ):
            # ---- constants ----
            ident = cpool.tile([128, 128], F32)
            make_identity(nc, ident[:])
            wf = cpool.tile([128, NCT, 128], F32)    # [c_lo, c_hi, Wq|Wk] f32
            wvf = cpool.tile([128, NCT, H], F32)
            nc.gpsimd.dma_start(
                wf[:, :, 0:H], wqD.rearrange("(k p) h -> p k h", p=128))
            nc.gpsimd.dma_start(
                wf[:, :, H:128], wkD.rearrange("(k p) h -> p k h", p=128))
            nc.gpsimd.dma_start(
                wvf[:], wvD.rearrange("(k p) h -> p k h", p=128))
            wqk = cpool.tile([128, NCT, 128], BF16)
            wv = cpool.tile([128, NCT, H], BF16)
            nc.vector.tensor_copy(wqk[:], wf[:])
            nc.vector.tensor_copy(wv[:], wvf[:])

            scrap = cpool.tile([128, 1], F32)
            # table preload: first Exp triggers ACT_TABLE_LOAD early
            nc.scalar.activation(scrap[:], ident[:, 0:1], AF.Exp)

            for rep in range(reps):
                emit_body(nc, tc, xD, outD,
                          (wqk, wv, ident),
                          (xnpool, xtpool, qkpool, ptpool, opool, fpool))

    nc.compile()
    return nc


def emit_body(nc, tc, xD, outD, consts, pools):
    AF = mybir.ActivationFunctionType
    ALU = mybir.AluOpType
    wqk, wv, ident = consts
    xnpool, xtpool, qkpool, ptpool, opool, fpool = pools

    x_nat = xnpool.tile([128, NTT, C], F32, tag="xnat")
    x_natb = xnpool.tile([128, NTT, C], BF16, tag="xnatb")
    xt = xtpool.tile([128, NCT, T], BF16, tag="xt")
    xR = xD.rearrange("(g p) c -> p g c", p=128)

    qk_a = qkpool.tile([128, T], BF16, tag="qka")   # rows 0:64 qT, 64:128 kT
    kq = qkpool.tile([128, T], BF16, tag="kq")      # rows 0:64 kT, 64:128 qT
    vt = qkpool.tile([64, T], BF16, tag="vt")
    v_nat = qkpool.tile([128, NTT, VP], BF16, tag="vnat")
    nc.gpsimd.memset(v_nat[:, :, H:H + 1], 1.0)
    o_out = fpool.tile([128, NTT, H], F32, tag="oout")
    outR = outD.rearrange("(g p) h -> p g h", p=128)

    def emit_piece(tk, cast_eng):
        nc.scalar.dma_start(x_nat[:, tk, :], xR[:, tk, :])
        if cast_eng == "act":
            nc.scalar.copy(x_natb[:, tk, :], x_nat[:, tk, :])
        else:
            nc.vector.tensor_copy(x_natb[:, tk, :], x_nat[:, tk, :])
        nc.sync.dma_start(
            xt[:, :, tk * 128:(tk + 1) * 128], x_natb[:, tk, :],
            transpose=True,
        )

    with (
        tc.tile_pool(name="qkps", bufs=1, space="PSUM") as qkps,
        tc.tile_pool(name="aux", bufs=1, space="PSUM") as aux,
        tc.tile_pool(name="ops", bufs=2, space="PSUM") as ops,
        tc.tile_pool(name="stps", bufs=2, space="PSUM") as stps,
    ):
        vps = fps = aux

        def emit_warm(n):
            # PE warm-up gated on the first casted tile; keeps HAM warm
            # through the load lead-in.
            warm = qkps.tile([128, 512], F32, tag="psqk")
            for _ in range(n):
                nc.tensor.matmul(
                    warm[:], x_natb[:, 0, 0:128], x_natb[:, 0, 0:512],
                    start=True, stop=True,
                )

        def emit_qkv(ci):
            sl = slice(ci * 512, (ci + 1) * 512)
            ps_v_t = vps.tile([128, 512], F32, tag="aux")
            ps_v = ps_v_t[0:64, :]
            for k in range(NCT):
                nc.tensor.matmul(
                    ps_v[:], wv[:, k, :], xt[:, k, sl],
                    start=(k == 0), stop=(k == NCT - 1),
                )
            nc.vector.tensor_copy(vt[:, sl], ps_v[:])
            nc.sync.dma_start(
                v_nat[:, ci * 4:(ci + 1) * 4, 0:H], vt[:, sl], transpose=True
            )
            ps_qk = qkps.tile([128, 512], F32, tag="psqk")
            for k in range(NCT):
                nc.tensor.matmul(
                    ps_qk[:], wqk[:, k, :], xt[:, k, sl],
                    start=(k == 0), stop=(k == NCT - 1),
                )
            nc.vector.tensor_copy(qk_a[:, sl], ps_qk[:])
            # kq: rows 0:64 <- kT (for even s-blocks, base 0);
            #     rows 64:128 <- qT duplicate (rhs for odd s-blocks, base 64)
            # SWDGE (gpsimd): SBUF->SBUF concurrent with xbar is safe there.
            nc.gpsimd.dma_start(kq[0:64, sl], qk_a[64:128, sl])
            nc.gpsimd.dma_start(kq[64:128, sl], qk_a[0:64, sl])

        out_pcs = {}

        def emit_attn_core(ci):
            out_pc = ops.tile([H + 1, 512], F32, tag="outc")
            out_pcs[ci] = out_pc
            npair = 2 * ci + 2
            nsb = 4 * ci + 4
            cl, cr = ci * 512, (ci + 1) * 512
            pending = []
            for p in range(npair):
                sbe, sbo = 2 * p, 2 * p + 1
                re, ro = sbe - 4 * ci, sbo - 4 * ci
                t0e, t0o = max(re, 0) * 128, max(ro, 0) * 128
                st = stps.tile([128, 1024], F32, tag="st")
                nc.tensor.matmul(
                    st[:, t0e:512],
                    kq[0:64, sbe * 128:(sbe + 1) * 128],
                    qk_a[0:64, cl + t0e:cr],
                    start=True, stop=True,
                )
                nc.tensor.matmul(
                    st[:, 512 + t0o:1024],
                    qk_a[64:128, sbo * 128:(sbo + 1) * 128],
                    kq[64:128, cl + t0o:cr],
                    start=True, stop=True,
                )
                pt = ptpool.tile([128, 1024], BF16, tag="pt")
                if re < 0:  # fully off-diagonal pair: one batched exp
                    nc.scalar.activation(
                        pt[:, 0:1024], st[:, 0:1024], AF.Exp, scale=SCALE)
                else:
                    nc.scalar.activation(
                        pt[:, t0e:512], st[:, t0e:512], AF.Exp, scale=SCALE)
                    nc.scalar.activation(
                        pt[:, 512 + t0o:1024], st[:, 512 + t0o:1024],
                        AF.Exp, scale=SCALE)
                    # zero upper triangle of the diagonal 128-blocks
                    nc.gpsimd.affine_select(
                        out=pt[:, t0e:t0e + 128], in_=pt[:, t0e:t0e + 128],
                        compare_op=ALU.is_ge, fill=0.0,
                        base=0, pattern=[[1, 128]], channel_multiplier=-1,
                    )
                    nc.gpsimd.affine_select(
                        out=pt[:, 512 + t0o:512 + t0o + 128],
                        in_=pt[:, 512 + t0o:512 + t0o + 128],
                        compare_op=ALU.is_ge, fill=0.0,
                        base=0, pattern=[[1, 128]], channel_multiplier=-1,
                    )
                if pending:
                    for args, kw in pending:
                        nc.tensor.matmul(*args, **kw)
                pending = [
                    ((out_pc[:, t0e:512], v_nat[:, sbe, 0:H + 1],
                      pt[:, t0e:512]),
                     dict(start=(sbe == 0), stop=False)),
                    ((out_pc[:, t0o:512], v_nat[:, sbo, 0:H + 1],
                      pt[:, 512 + t0o:1024]),
                     dict(start=False, stop=(sbo == nsb - 1))),
                ]
            for args, kw in pending:
                nc.tensor.matmul(*args, **kw)

        def emit_attn_out(ci):
            out_pc = out_pcs[ci]
            o_c = opool.tile([H + 1, 512], F32, tag="osb")
            nc.vector.tensor_copy(o_c[:], out_pc[:])
            fin_t = fps.tile([128, 4, 128], F32, tag="aux")
            fin4 = fin_t[:, :, 0:H + 1]
            for rr in range(4):
                nc.tensor.transpose(
                    fin4[:, rr, :],
                    o_c[:, rr * 128:(rr + 1) * 128],
                    ident[0:H + 1, 0:H + 1],
                )
            rcp = fpool.tile([128, 4, 1], F32, tag="rcp")
            nc.vector.reciprocal(rcp[:], fin4[:, :, H:H + 1])
            for rr in range(4):
                tk = ci * 4 + rr
                nc.vector.tensor_scalar_mul(
                    o_out[:, tk, :], fin4[:, rr, 0:H], rcp[:, rr, :]
                )
            nc.scalar.dma_start(
                outR[:, ci * 4:(ci + 1) * 4, :],
                o_out[:, ci * 4:(ci + 1) * 4, :],
            )

        # ---- software-pipelined emission ----
        ACT_CAST_TILES = (1, 3)   # these tiles cast on ScalarE, rest on DVE
        def pieces(lo, hi):
            for tk in range(lo, hi):
                emit_piece(tk, "act" if tk in ACT_CAST_TILES else "dve")

        pieces(0, 2)
        emit_warm(10)
        pieces(2, 4)
        emit_qkv(0)
        pieces(4, 6)
        emit_attn_core(0)
        pieces(6, 8)
        emit_qkv(1)
        emit_attn_out(0)
        pieces(8, 10)
        emit_attn_core(1)
        pieces(10, 12)
        emit_qkv(2)
        emit_attn_out(1)
        pieces(12, 14)
        emit_attn_core(2)
        pieces(14, 16)
        emit_qkv(3)
        emit_attn_out(2)
        emit_attn_core(3)
        emit_attn_out(3)


_NC = None


def kernel(x, Wq, Wk, Wv):
    global _NC
    if _NC is None:
        _NC = build_nc()
    in_maps = [
        {
            "x": np.ascontiguousarray(x[b], dtype=np.float32),
            "Wq": np.ascontiguousarray(Wq, dtype=np.float32),
            "Wk": np.ascontiguousarray(Wk, dtype=np.float32),
            "Wv": np.ascontiguousarray(Wv, dtype=np.float32),
        }
        for b in range(B)
    ]
    res = run_bass_kernel_spmd(_NC, in_maps, core_ids=list(range(B)))
    return np.stack([res.results[b]["out"] for b in range(B)], axis=0)


# revision 3
# speedup vs baseline: 1.1305x; 1.1305x over previous
"""Single-head causal attention on 8 TRN2 NeuronCores — v3 (streamed load).

Problem: x[B=8, T=2048, C=1024], Wq/Wk/Wv[C, H=64] (fp32)
  q = x@Wq; k = x@Wk; v = x@Wv
  wei = softmax(mask(q k^T * C^-0.5)); out = wei @ v       -> [B, T, H]

Sharding: data-parallel over batch, one batch element per core.

Per-core dataflow:
  - x loaded fp32 in 8x 1MB pieces on the scalar HWDGE queue (all queued
    up-front so the HW ring streams them back-to-back at HBM rate),
    cast fp32->bf16 per t-tile on DVE (2x mode), xbar-transposed per
    t-tile on the sync queue.  QKV chunk 0 starts ~9us in, overlapping
    the remaining load.
  - S^T row-packed pairs: even s-blocks at partitions 0:64 (kT copied to
    base 0), odd s-blocks at partitions 64:128 (kT in situ in qk_a +
    duplicated qT at base 64) -> 2 concurrent K=64 matmuls.
  - exp batched per pair [128,1024] across 2 PSUM banks (off-diagonal).
  - causal mask = post-exp affine_select zeroing on bf16 pt (gpsimd).
  - PV accumulates [v|1]^T @ exp(S^T) -> row 64 gives sumexp for free;
    PE-transpose + reciprocal + scale for the final [T,H] output.
"""
import sys

sys.path.insert(0, "/opt/trn_rl_repo")

import numpy as np

import concourse.bass as bass
import concourse.mybir as mybir
import concourse.tile as tile
from concourse import bacc
from concourse.bass_utils import run_bass_kernel_spmd
from concourse.masks import make_identity

B, T, C, H = 8, 2048, 1024, 64
NTT = T // 128   # 16 t-tiles
NCT = C // 128   # 8  c-tiles
NCH = T // 512   # 4  t-chunks
SCALE = float(C) ** -0.5
VP = 80          # v_nat per-tile stride: 160B, 32B-aligned for xbar transpose

F32 = mybir.dt.float32
BF16 = mybir.dt.bfloat16


def build_nc(reps=1):
    nc = bacc.Bacc("TRN2", target_bir_lowering=False, debug=False)
    xD = nc.dram_tensor("x", [T, C], F32, kind="ExternalInput").ap()
    wqD = nc.dram_tensor("Wq", [C, H], F32, kind="ExternalInput").ap()
    wkD = nc.dram_tensor("Wk", [C, H], F32, kind="ExternalInput").ap()
    wvD = nc.dram_tensor("Wv", [C, H], F32, kind="ExternalInput").ap()
    outD = nc.dram_tensor("out", [T, H], F32, kind="ExternalOutput").ap()

    AF = mybir.ActivationFunctionType

    with tile.TileContext(nc) as tc:
        with (
            tc.tile_pool(name="const", bufs=1) as cpool,
            tc.tile_pool(name="xnat", bufs=1) as xnpool,
            tc.tile_pool(name="xt", bufs=1) as xtpool,
            tc.tile_pool(name="qk", bufs=1) as qkpool,
            tc.tile_pool(name="pt", bufs=4) as ptpool,
            tc.tile_pool(name="osb", bufs=3) as opool,
            tc.tile_pool(name="fin", bufs=2) as fpool,
        ):
            # ---- constants ----
            ident = cpool.tile([128, 128], F32)
            make_identity(nc, ident[:])
            wf = cpool.tile([128, NCT, 128], F32)    # [c_lo, c_hi, Wq|Wk] f32
            wvf = cpool.tile([128, NCT, H], F32)
            nc.gpsimd.dma_start(
                wf[:, :, 0:H], wqD.rearrange("(k p) h -> p k h", p=128))
            nc.gpsimd.dma_start(
                wf[:, :, H:128], wkD.rearrange("(k p) h -> p k h", p=128))
            nc.gpsimd.dma_start(
                wvf[:], wvD.rearrange("(k p) h -> p k h", p=128))
            wqk = cpool.tile([128, NCT, 128], BF16)
            wv = cpool.tile([128, NCT, H], BF16)
            nc.vector.tensor_copy(wqk[:], wf[:])
            nc.vector.tensor_copy(wv[:], wvf[:])

            scrap = cpool.tile([128, 1], F32)
            # table preload: first Exp triggers ACT_TABLE_LOAD early
            nc.scalar.activation(scrap[:], ident[:, 0:1], AF.Exp)

            for rep in range(reps):
                emit_body(nc, tc, xD, outD,
                          (wqk, wv, ident),
                          (xnpool, xtpool, qkpool, ptpool, opool, fpool))

    nc.compile()
    return nc


def emit_body(nc, tc, xD, outD, consts, pools):
    AF = mybir.ActivationFunctionType
    ALU = mybir.AluOpType
    wqk, wv, ident = consts
    xnpool, xtpool, qkpool, ptpool, opool, fpool = pools

    x_nat = xnpool.tile([128, NTT, C], F32, tag="xnat")
    x_natb = xnpool.tile([128, NTT, C], BF16, tag="xnatb")
    xt = xtpool.tile([128, NCT, T], BF16, tag="xt")
    xR = xD.rearrange("(g p) c -> p g c", p=128)

    qk_a = qkpool.tile([128, T], BF16, tag="qka")   # rows 0:64 qT, 64:128 kT
    kq = qkpool.tile([128, T], BF16, tag="kq")      # rows 0:64 kT, 64:128 qT
    vt = qkpool.tile([64, T], BF16, tag="vt")
    v_nat = qkpool.tile([128, NTT, VP], BF16, tag="vnat")
    nc.gpsimd.memset(v_nat[:, :, H:H + 1], 1.0)
    o_out = fpool.tile([128, NTT, H], F32, tag="oout")
    outR = outD.rearrange("(g p) h -> p g h", p=128)

    def castr(tk):
        # cast piece's t-tile on DVE, then xbar-transpose it on sync
        nc.vector.tensor_copy(x_natb[:, tk, :], x_nat[:, tk, :])
        nc.sync.dma_start(
            xt[:, :, tk * 128:(tk + 1) * 128], x_natb[:, tk, :],
            transpose=True,
        )

    with (
        tc.tile_pool(name="qkps", bufs=1, space="PSUM") as qkps,
        tc.tile_pool(name="aux", bufs=1, space="PSUM") as aux,
        tc.tile_pool(name="ops", bufs=2, space="PSUM") as ops,
        tc.tile_pool(name="stps", bufs=2, space="PSUM") as stps,
    ):
        vps = fps = aux

        def emit_warm(n):
            # PE warm-up gated on the first casted tile; keeps HAM warm
            # through the load lead-in.
            warm = qkps.tile([128, 512], F32, tag="psqk")
            for _ in range(n):
                nc.tensor.matmul(
                    warm[:], x_natb[:, 0, 0:128], x_natb[:, 0, 0:512],
                    start=True, stop=True,
                )

        def emit_qkv(ci):
            sl = slice(ci * 512, (ci + 1) * 512)
            ps_v_t = vps.tile([128, 512], F32, tag="aux")
            ps_v = ps_v_t[0:64, :]
            for k in range(NCT):
                nc.tensor.matmul(
                    ps_v[:], wv[:, k, :], xt[:, k, sl],
                    start=(k == 0), stop=(k == NCT - 1),
                )
            nc.vector.tensor_copy(vt[:, sl], ps_v[:])
            nc.sync.dma_start(
                v_nat[:, ci * 4:(ci + 1) * 4, 0:H], vt[:, sl], transpose=True
            )
            ps_qk = qkps.tile([128, 512], F32, tag="psqk")
            for k in range(NCT):
                nc.tensor.matmul(
                    ps_qk[:], wqk[:, k, :], xt[:, k, sl],
                    start=(k == 0), stop=(k == NCT - 1),
                )
            nc.vector.tensor_copy(qk_a[:, sl], ps_qk[:])
            # kq: rows 0:64 <- kT (lhsT for even s-blocks, base 0);
            #     rows 64:128 <- qT duplicate (rhs for odd s-blocks, base 64)
            # SWDGE (gpsimd): SBUF->SBUF concurrent with xbar is safe there.
            nc.gpsimd.dma_start(kq[0:64, sl], qk_a[64:128, sl])
            nc.gpsimd.dma_start(kq[64:128, sl], qk_a[0:64, sl])

        out_pcs = {}

        def emit_attn_core(ci):
            out_pc = ops.tile([H + 1, 512], F32, tag="outc")
            out_pcs[ci] = out_pc
            npair = 2 * ci + 2
            nsb = 4 * ci + 4
            cl, cr = ci * 512, (ci + 1) * 512
            pending = []
            for p in range(npair):
                sbe, sbo = 2 * p, 2 * p + 1
                re, ro = sbe - 4 * ci, sbo - 4 * ci
                t0e, t0o = max(re, 0) * 128, max(ro, 0) * 128
                st = stps.tile([128, 1024], F32, tag="st")
                nc.tensor.matmul(
                    st[:, t0e:512],
                    kq[0:64, sbe * 128:(sbe + 1) * 128],
                    qk_a[0:64, cl + t0e:cr],
                    start=True, stop=True,
                )
                nc.tensor.matmul(
                    st[:, 512 + t0o:1024],
                    qk_a[64:128, sbo * 128:(sbo + 1) * 128],
                    kq[64:128, cl + t0o:cr],
                    start=True, stop=True,
                )
                pt = ptpool.tile([128, 1024], BF16, tag="pt")
                if re < 0:  # fully off-diagonal pair: one batched exp
                    nc.scalar.activation(
                        pt[:, 0:1024], st[:, 0:1024], AF.Exp, scale=SCALE)
                else:
                    nc.scalar.activation(
                        pt[:, t0e:512], st[:, t0e:512], AF.Exp, scale=SCALE)
                    nc.scalar.activation(
                        pt[:, 512 + t0o:1024], st[:, 512 + t0o:1024],
                        AF.Exp, scale=SCALE)
                    # zero upper triangle of the diagonal 128-blocks
                    nc.gpsimd.affine_select(
                        out=pt[:, t0e:t0e + 128], in_=pt[:, t0e:t0e + 128],
                        compare_op=ALU.is_ge, fill=0.0,
                        base=0, pattern=[[1, 128]], channel_multiplier=-1,
                    )
                    nc.gpsimd.affine_select(
                        out=pt[:, 512 + t0o:512 + t0o + 128],
                        in_=pt[:, 512 + t0o:512 + t0o + 128],
                        compare_op=ALU.is_ge, fill=0.0,
                        base=0, pattern=[[1, 128]], channel_multiplier=-1,
                    )
                if pending:
                    for args, kw in pending:
                        nc.tensor.matmul(*args, **kw)
                pending = [
                    ((out_pc[:, t0e:512], v_nat[:, sbe, 0:H + 1],
                      pt[:, t0e:512]),
                     dict(start=(sbe == 0), stop=False)),
                    ((out_pc[:, t0o:512], v_nat[:, sbo, 0:H + 1],
                      pt[:, 512 + t0o:1024]),
                     dict(start=False, stop=(sbo == nsb - 1))),
                ]
            for args, kw in pending:
                nc.tensor.matmul(*args, **kw)

        def emit_attn_out(ci):
            out_pc = out_pcs[ci]
            o_c = opool.tile([H + 1, 512], F32, tag="osb")
            nc.vector.tensor_copy(o_c[:], out_pc[:])
            fin_t = fps.tile([128, 4, 128], F32, tag="aux")
            fin4 = fin_t[:, :, 0:H + 1]
            for rr in range(4):
                nc.tensor.transpose(
                    fin4[:, rr, :],
                    o_c[:, rr * 128:(rr + 1) * 128],
                    ident[0:H + 1, 0:H + 1],
                )
            rcp = fpool.tile([128, 4, 1], F32, tag="rcp")
            nc.vector.reciprocal(rcp[:], fin4[:, :, H:H + 1])
            for rr in range(4):
                tk = ci * 4 + rr
                nc.vector.tensor_scalar_mul(
                    o_out[:, tk, :], fin4[:, rr, 0:H], rcp[:, rr, :]
                )
            nc.gpsimd.dma_start(
                outR[:, ci * 4:(ci + 1) * 4, :],
                o_out[:, ci * 4:(ci + 1) * 4, :],
            )

        # ---- queue all 8 x-piece loads up-front: the scalar HWDGE ring
        # streams them back-to-back at HBM rate, sems fire per-piece ----
        for p in range(8):
            nc.scalar.dma_start(x_nat[:, 2 * p:2 * p + 2, :],
                                xR[:, 2 * p:2 * p + 2, :])

        castr(0)
        castr(1)
        emit_warm(8)
        castr(2)
        castr(3)
        emit_qkv(0)
        castr(4)
        castr(5)
        emit_attn_core(0)
        castr(6)
        castr(7)
        emit_qkv(1)
        emit_attn_out(0)
        castr(8)
        castr(9)
        emit_attn_core(1)
        castr(10)
        castr(11)
        emit_qkv(2)
        emit_attn_out(1)
        castr(12)
        castr(13)
        emit_attn_core(2)
        castr(14)
        castr(15)
        emit_qkv(3)
        emit_attn_out(2)
        emit_attn_core(3)
        emit_attn_out(3)


_NC = None


def kernel(x, Wq, Wk, Wv):
    global _NC
    if _NC is None:
        _NC = build_nc()
    in_maps = [
        {
            "x": np.ascontiguousarray(x[b], dtype=np.float32),
            "Wq": np.ascontiguousarray(Wq, dtype=np.float32),
            "Wk": np.ascontiguousarray(Wk, dtype=np.float32),
            "Wv": np.ascontiguousarray(Wv, dtype=np.float32),
        }
        for b in range(B)
    ]
    res = run_bass_kernel_spmd(_NC, in_maps, core_ids=list(range(B)))
    return np.stack([res.results[b]["out"] for b in range(B)], axis=0)
